# revision 1
# baseline (speedup 1.0000x reference)
"""Trainium2 Bass kernel for sliding-window Pearson correlation attention.

Input  x: [512, 2, 32768] f32.
Output attentions: [512, 32669] f32 = relu(corr - mean_b(corr)) where corr is
the per-batch sliding-window (w=100) Pearson correlation of the two channels.

Sharding: split the T/output dimension across the 8 cores (4084 output
columns each, + 99-column halo on the input). Every core sees all 512
batches, so the batch-mean is computed locally - no collective needed.

Layout: batch-major (partition = batch row, 4 tiles of 128). Windowed sums
are computed with the DVE scan instruction via the recurrence
    s[i+1] = s[i] + a[i+100] - a[i]
(one streaming pass per sequence, all 5 sequences pre-scaled by w so the
Pearson formula reduces to plain tensor-tensor ops). Squares and
rsqrt (exp(-0.5*ln)) run on ScalarE, three of the elementwise products on
GpSimd, the batch mean + partition broadcast on the PE (ones-matmuls), and
the variance subtract reads its second operand from PSUM to keep the shared
DVE/GpSimd SBUF port free.
"""

import numpy as np

import concourse.bass as bass
import concourse.mybir as mybir
import concourse.tile as tile
from concourse.bass_utils import run_bass_kernel_spmd

WIN = 100
B = 512
CH = 2
T = 32768
N = T - WIN + 1  # 32669
NCORES = 8
NLOC = 4084  # output columns per core (8*4084 = 32672 >= N; tail dropped)
FIN = NLOC + WIN - 1  # 4183 input columns per core
TPAD = (NCORES - 1) * NLOC + FIN  # 32771 (input padded with 3 zero cols)
P = 128
NBT = B // P  # 4 batch tiles
NCHUNK = 4
F = NLOC // NCHUNK  # 1021 output columns per chunk
H = F + WIN - 1  # 1120 input columns per chunk

f32 = mybir.dt.float32
AOT = mybir.ActivationFunctionType
ALU = mybir.AluOpType
AXL = mybir.AxisListType

REPEAT = 1  # bench-only: repeat the whole computation inside one NEFF


def _kernel_body(tc, out, xs):
    nc = tc.nc
    import contextlib

    ctx = contextlib.ExitStack()
    with ctx:
        const_pool = ctx.enter_context(tc.tile_pool(name="const", bufs=1))
        pool = ctx.enter_context(tc.tile_pool(name="work", bufs=3))
        corr_pool = ctx.enter_context(tc.tile_pool(name="corrp", bufs=6))
        row_pool = ctx.enter_context(tc.tile_pool(name="rows", bufs=2))
        psum_pool = ctx.enter_context(tc.tile_pool(name="psum", bufs=2, space="PSUM"))

        ones = const_pool.tile([P, 1], f32, tag="ones")
        nc.vector.memset(ones[:], 1.0)
        ones_row = const_pool.tile([1, P], f32, tag="ones_row")
        nc.vector.memset(ones_row[:], 1.0)

        NEG_INV_W = -1.0 / WIN
        NEG_INV_B = -1.0 / B

        SQW = float(np.sqrt(WIN))

        def wsum(dst2d, src2d):
            # dst[:, i] = sum(src[:, i:i+WIN]) for i in [0, F)
            # first-window sum via reduce, the rest via the DVE scan
            # recurrence s[i+1] = s[i] + a[i+w] - a[i].
            nc.vector.reduce_sum(dst2d[:, 0:1], src2d[:, 0:WIN], axis=AXL.X)
            nc.vector.tensor_tensor_scan(
                out=dst2d[:, 1:F],
                data0=src2d[:, WIN : WIN + F - 1],
                data1=src2d[:, 0 : F - 1],
                initial=dst2d[:, 0:1],
                op0=ALU.add,
                op1=ALU.subtract,
            )

        SPLIT = min(512, F)
        for c in range(NCHUNK * REPEAT):
            c = c % NCHUNK
            c0 = c * F
            psA = psum_pool.tile([1, SPLIT], f32, tag="psA", bufs=1)
            psB = (
                psum_pool.tile([1, F - SPLIT], f32, tag="psB", name="psB", bufs=1)
                if F > SPLIT
                else None
            )
            corrs = []
            for bt in range(NBT):
                b0 = bt * P
                x12 = pool.tile([P, CH, H], f32, tag="x12")
                nc.sync.dma_start(out=x12[:], in_=xs[b0 : b0 + P, :, c0 : c0 + H])
                x1 = x12[:, 0, :]
                x2 = x12[:, 1, :]

                # all quantities in w-scaled units: e = w*x^2, e12w = w*x1*x2
                e = pool.tile([P, CH, H], f32, tag="e")
                nc.scalar.activation(e[:], x12[:], AOT.Square, scale=SQW)
                x1s = pool.tile([P, H], f32, tag="x1s")
                nc.scalar.mul(x1s[:], x1, float(WIN))
                e12 = pool.tile([P, H], f32, tag="e12")
                nc.gpsimd.tensor_mul(e12[:], x1s[:], x2)

                s = pool.tile([P, CH, F], f32, tag="s")  # s1, s2
                se = pool.tile([P, CH, F], f32, tag="se")  # w*s11, w*s22
                # w*s12 scan lands in PSUM so the cov subtract reads it via
                # the PSUM port (SBUF port 1 stays free for GpSimd)
                s12 = psum_pool.tile([P, F], f32, tag="s12", bufs=1)
                wsum(s[:, 0, :], x1)
                wsum(s[:, 1, :], x2)
                wsum(se[:, 0, :], e[:, 0, :])
                wsum(se[:, 1, :], e[:, 1, :])
                wsum(s12[:], e12[:])

                # v = w*s11 - s1^2, channel-split so t needs only 2 PSUM banks
                t = psum_pool.tile([P, F], f32, tag="t", bufs=1)
                nc.scalar.activation(t[:], s[:, 0, :], AOT.Square)
                nc.vector.tensor_sub(se[:, 0, :], se[:, 0, :], t[:])
                t2 = psum_pool.tile([P, F], f32, tag="t", name="t2", bufs=1)
                nc.scalar.activation(t2[:], s[:, 1, :], AOT.Square)
                nc.vector.tensor_sub(se[:, 1, :], se[:, 1, :], t2[:])
                # cov = w*s12 - s1*s2
                t12 = pool.tile([P, F], f32, tag="t12")
                nc.gpsimd.tensor_mul(t12[:], s[:, 0, :], s[:, 1, :])
                cov = pool.tile([P, F], f32, tag="cov")
                nc.vector.tensor_sub(cov[:], s12[:], t12[:])
                # corr = cov * rsqrt(v1*v2);  rsqrt via exp(-0.5*ln)
                p = pool.tile([P, F], f32, tag="p")
                nc.gpsimd.tensor_mul(p[:], se[:, 0, :], se[:, 1, :])
                nc.scalar.activation(p[:], p[:], AOT.Ln)
                # rs lands in PSUM (shares the t banks - t is dead by now) so
                # the corr multiply reads via the PSUM port, leaving SBUF
                # port 1 free for the GpSimd products.
                rs = psum_pool.tile([P, F], f32, tag="t", name="rs", bufs=1)
                nc.scalar.activation(rs[:], p[:], AOT.Exp, scale=-0.5)
                corr = corr_pool.tile([P, F], f32, tag="corr")
                nc.vector.tensor_mul(corr[:], cov[:], rs[:])
                corrs.append(corr)

                # batch-sum via ones-matmul (accumulate over the 4 batch tiles)
                nc.tensor.matmul(
                    psA[:], ones[:], corr[:, 0:SPLIT],
                    start=(bt == 0), stop=(bt == NBT - 1),
                )
                if psB is not None:
                    nc.tensor.matmul(
                        psB[:], ones[:], corr[:, SPLIT:F],
                        start=(bt == 0), stop=(bt == NBT - 1),
                    )

            # -mean row (negate+scale while copying PSUM->SBUF)
            avg_row = row_pool.tile([1, F], f32, tag="avgrow")
            nc.scalar.mul(avg_row[:, 0:SPLIT], psA[:], NEG_INV_B)
            if psB is not None:
                nc.scalar.mul(avg_row[:, SPLIT:F], psB[:], NEG_INV_B)
            # broadcast -mean to all partitions via K=1 matmul, stage to SBUF
            avgb = psum_pool.tile([P, F], f32, tag="avgb", bufs=1)
            nc.tensor.matmul(avgb[:, 0:SPLIT], ones_row[:], avg_row[:, 0:SPLIT])
            if F > SPLIT:
                nc.tensor.matmul(avgb[:, SPLIT:F], ones_row[:], avg_row[:, SPLIT:F])
            for bt in range(NBT):
                b0 = bt * P
                corr = corrs[bt]
                nc.vector.tensor_add(corr[:], corr[:], avgb[:])
                nc.scalar.activation(corr[:], corr[:], AOT.Relu)
                nc.sync.dma_start(out=out[b0 : b0 + P, c0 : c0 + F], in_=corr[:])


def build_nc():
    from concourse import bacc

    nc = bacc.Bacc("TRN2", target_bir_lowering=False, debug=False, num_devices=NCORES)
    xs = nc.dram_tensor("xs", [B, CH, FIN], f32, kind="ExternalInput").ap()
    out = nc.dram_tensor("out", [B, NLOC], f32, kind="ExternalOutput").ap()
    with tile.TileContext(nc) as tc:
        _kernel_body(tc, out, xs)
    nc.compile()
    return nc


_NC = None


def _get_nc():
    global _NC
    if _NC is None:
        _NC = build_nc()
    return _NC


def make_in_maps(x):
    x = np.asarray(x, dtype=np.float32)
    xpad = np.zeros((B, CH, TPAD), dtype=np.float32)
    xpad[:, :, :T] = x
    return [
        {"xs": np.ascontiguousarray(xpad[:, :, c * NLOC : c * NLOC + FIN])}
        for c in range(NCORES)
    ]


def _run(x, **kwargs):
    nc = _get_nc()
    res = run_bass_kernel_spmd(nc, make_in_maps(x), core_ids=list(range(NCORES)), **kwargs)
    outs = [res.results[c]["out"] for c in range(NCORES)]
    full = np.concatenate(outs, axis=1)[:, :N].astype(np.float32)
    return full, res


def kernel(x):
    full, _ = _run(x)
    return full



# revision 12
# speedup vs baseline: 1.4725x; 1.4725x over previous
"""Trainium2 Bass kernel for sliding-window Pearson correlation attention.

Input  x: [512, 2, 32768] f32.
Output attentions: [512, 32669] f32 = relu(corr - mean_b(corr)) where corr is
the per-batch sliding-window (w=100) Pearson correlation of the two channels.

Sharding: split the T/output dimension across the 8 cores (4084 output
columns each, + 99-column halo on the input). Every core sees all 512
batches, so the batch-mean is computed locally - no collective needed.

Pipeline (per core, per 128-batch tile):
  scans   DVE   5 windowed sums via tensor_tensor_scan (f32 data in,
                bf16 out); first window seeded by a 99-col reduce through a
                leading zero column; second T-chunk chains off the first.
  squares Act   e = x^2 (bf16), t = (s/sqrt(w))^2, sqrt of the variances
  e12     Pool  x1*x2 via scalar_tensor_tensor (cheaper than tensor_tensor
                in the cost model)
  u       DVE   s1*s2 bf16 tensor_tensor (2x mode)
  v1,v2   PE    identity-matmul accumulate: v = I@s11 - I@t1 into PSUM
                (weights are free, so the subtract rides the matmul)
  cov,den Pool  scalar_tensor_tensor fusions
  corr    P/D   cov / den (bf16 divide, split across engines to balance)
  mean    PE    (1/B)ones-matmul accumulated over the 4 batch tiles
  fsub    PE    I@corr - broadcast(avg) into PSUM
  relu    A/P   psum -> sbuf bf16, DMA out (f32 recovered on host)
"""

import numpy as np

import concourse.bass as bass
import concourse.mybir as mybir
import concourse.tile as tile
from concourse.bass_utils import run_bass_kernel_spmd

WIN = 100
B = 512
CH = 2
T = 32768
N = T - WIN + 1  # 32669
NCORES = 8
NLOC = 4084  # output columns per core (8*4084 = 32672 >= N; tail dropped)
FIN = NLOC + WIN - 1  # 4183 input columns per core
TPAD = (NCORES - 1) * NLOC + FIN  # 32771 (input padded with 3 zero cols)
P = 128
NBT = B // P  # 4 batch tiles

CHUNKS = [1021, 2042, 1021]  # scan chunk widths along T (sum = NLOC)
assert sum(CHUNKS) == NLOC
COFF = [sum(CHUNKS[:i]) for i in range(len(CHUNKS))]
NSC = len(CHUNKS)
FSMAX = max(CHUNKS)
WTMAX = FSMAX + WIN + 1  # input cols per scan chunk (incl. leading zero/halo)

SL = 512  # psum-bank slice for matmuls
NSL = (NLOC + SL - 1) // SL  # 8 slices (last = 500)

f32 = mybir.dt.float32
bf16 = mybir.dt.bfloat16
AOT = mybir.ActivationFunctionType
ALU = mybir.AluOpType
AXL = mybir.AxisListType

# engine-split knobs (fraction of columns handled by the DVE engine; the
# rest goes to Pool for SBUF ops / Act for PSUM-reading relu)
U_DVE = 0.0      # u = s1*s2
M0_DVE = 0.0     # m0 = r1*r2
COV_DVE = 1.0    # cov = s12w - u
CORR_DVE = 1.0   # corr = cov*m0
RELU_ACT = 1.0   # relu: Act share (rest DVE tensor_scalar_max; Pool can't read PSUM)
PSL = 1021       # slice width for Pool/DVE post ops (latency hiding)


def _slices(total, step):
    return [(i, min(i + step, total)) for i in range(0, total, step)]


def _kernel_body(tc, out, xs, wconst):
    nc = tc.nc
    import contextlib

    ctx = contextlib.ExitStack()
    with ctx:
        const_pool = ctx.enter_context(tc.tile_pool(name="const", bufs=1))
        in_pool = ctx.enter_context(tc.tile_pool(name="scanin", bufs=2))
        s_pool = ctx.enter_context(tc.tile_pool(name="scanout", bufs=2))
        post_pool = ctx.enter_context(tc.tile_pool(name="post", bufs=2))
        corr_pool = ctx.enter_context(tc.tile_pool(name="corrp", bufs=2))
        row_pool = ctx.enter_context(tc.tile_pool(name="rows", bufs=2))
        out_pool = ctx.enter_context(tc.tile_pool(name="outp", bufs=2))
        v_psum = ctx.enter_context(tc.tile_pool(name="vps", bufs=2, space="PSUM"))
        b_psum = ctx.enter_context(tc.tile_pool(name="bps", bufs=1, space="PSUM"))
        f_psum = ctx.enter_context(tc.tile_pool(name="fps", bufs=3, space="PSUM"))

        # constants: I and -I (bf16, exact)
        identP = const_pool.tile([P, P], bf16, tag="identP")
        nc.sync.dma_start(out=identP[:], in_=wconst[0, :, :])
        identN = const_pool.tile([P, P], bf16, tag="identN")
        nc.sync.dma_start(out=identN[:], in_=wconst[1, :, :])
        ident = [identP, identN]
        bcol = const_pool.tile([P, 1], bf16, tag="bcol")
        nc.vector.memset(bcol[:], 1.0 / B)
        negrow = const_pool.tile([1, P], bf16, tag="negrow")
        nc.vector.memset(negrow[:], -1.0)

        SQW = float(np.sqrt(WIN))

        # column-major: each T-chunk is fully independent (scan seeds come
        # from a reduce over the chunk's own halo). The (c, bt) tasks are
        # software-pipelined with a one-stage skew so every engine queue sees
        # task k+1's scan-stage ops before task k's post-stage ops — the
        # in-order queues never stall behind the cross-engine post chain.
        def scan_stage(c, bt):
            fs = CHUNKS[c]
            c0 = COFF[c]
            wt = fs + WIN + 1
            b0 = bt * P
            x12z = in_pool.tile([P, CH, WTMAX], f32, tag="x12z")
            if c == 0:
                # leading zero column so data1[0] reads 0 at t=0
                nc.vector.memset(x12z[:, :, 0:1], 0.0)
                nc.sync.dma_start(
                    out=x12z[:, :, 1:wt],
                    in_=xs[b0 : b0 + P, :, 0 : wt - 1],
                )
            else:
                # halo: col j holds x[c0-1+j]
                nc.sync.dma_start(
                    out=x12z[:, :, 0 : wt - 1],
                    in_=xs[b0 : b0 + P, :, c0 - 1 : c0 - 1 + wt - 1],
                )
            e = in_pool.tile([P, CH, WTMAX], bf16, tag="e")
            nc.scalar.activation(e[:, :, 0:wt], x12z[:, :, 0:wt], AOT.Square, scale=SQW)
            x1s = in_pool.tile([P, WTMAX], bf16, tag="x1s")
            nc.scalar.mul(x1s[:, 0:wt], x12z[:, 0, 0:wt], float(WIN))
            e12 = in_pool.tile([P, WTMAX], bf16, tag="e12")
            nc.gpsimd.tensor_mul(e12[:, 0:wt], x1s[:, 0:wt], x12z[:, 1, 0:wt])

            s_pair = s_pool.tile([P, CH, FSMAX], bf16, tag="s_pair")
            se_pair = s_pool.tile([P, CH, FSMAX], bf16, tag="se_pair")
            s12 = s_pool.tile([P, FSMAX], bf16, tag="s12")

            def wsum(dst, src, ch, tg):
                # dst[:, t] = windowed sum at c0+t; src col j = a[c0-1+j]
                # (j=0 is a zero col for chunk 0). Seed = window at t-1,
                # reduced over the halo cols [0, WIN).
                if ch is not None:
                    d0, d1, rs = (
                        src[:, ch, WIN : WIN + fs],
                        src[:, ch, 0:fs],
                        src[:, ch, 0:WIN],
                    )
                else:
                    d0, d1, rs = (
                        src[:, WIN : WIN + fs],
                        src[:, 0:fs],
                        src[:, 0:WIN],
                    )
                init = in_pool.tile([P, 1], f32, tag=f"init_{tg}")
                nc.vector.reduce_sum(init[:], rs, axis=AXL.X)
                nc.vector.tensor_tensor_scan(
                    out=dst,
                    data0=d0,
                    data1=d1,
                    initial=init[:],
                    op0=ALU.add,
                    op1=ALU.subtract,
                )

            wsum(s_pair[:, 0, 0:fs], x12z, 0, "s1")
            wsum(s_pair[:, 1, 0:fs], x12z, 1, "s2")
            wsum(se_pair[:, 0, 0:fs], e, 0, "se1")
            wsum(se_pair[:, 1, 0:fs], e, 1, "se2")
            wsum(s12[:, 0:fs], e12, None, "s12")
            return s_pair, se_pair, s12

        def split_op(dve_emit, pool_emit, frac, fs):
            # column-split an elementwise op: [0, cut) on DVE, [cut, fs) Pool
            cut = int(fs * frac) // 2 * 2
            for (l, r) in _slices(fs, PSL):
                dl, dr = min(l, cut), min(r, cut)
                if dl < dr:
                    dve_emit(dl, dr)
                pl, pr = max(l, cut), max(r, cut)
                if pl < pr:
                    pool_emit(pl, pr)

        def post_stage(c, bt, scans):
            fs = CHUNKS[c]
            s_pair, se_pair, s12 = scans
            # t = s^2 (plain; the w-scaling lives in e/e12)
            t_pair = post_pool.tile([P, CH, FSMAX], bf16, tag="t_pair")
            nc.scalar.activation(
                t_pair[:, :, 0:fs], s_pair[:, :, 0:fs], AOT.Square
            )
            u = post_pool.tile([P, FSMAX], bf16, tag="u")
            split_op(
                lambda l, r: nc.vector.tensor_mul(
                    u[:, l:r], s_pair[:, 0, l:r], s_pair[:, 1, l:r]
                ),
                lambda l, r: nc.gpsimd.tensor_mul(
                    u[:, l:r], s_pair[:, 0, l:r], s_pair[:, 1, l:r]
                ),
                U_DVE, fs,
            )

            # v = w*s11 - s1^2 (PE identity matmuls into PSUM), rsqrt on Act
            r_pair = post_pool.tile([P, CH, FSMAX], bf16, tag="r_pair")
            for (l, r) in _slices(fs, SL):
                vps = v_psum.tile([P, CH, SL], f32, tag="vps")
                for ch in range(CH):
                    nc.tensor.matmul(
                        vps[:, ch, 0 : r - l], ident[0][:], se_pair[:, ch, l:r],
                        start=True, stop=False,
                    )
                    nc.tensor.matmul(
                        vps[:, ch, 0 : r - l], ident[1][:], t_pair[:, ch, l:r],
                        start=False, stop=True,
                    )
                nc.scalar.activation(
                    r_pair[:, :, l:r], vps[:, :, 0 : r - l], AOT.Abs_reciprocal_sqrt
                )

            # cov = w*s12 - u ; m0 = r1*r2 ; corr = cov*m0
            cov = post_pool.tile([P, FSMAX], bf16, tag="cov")
            split_op(
                lambda l, r: nc.vector.tensor_sub(cov[:, l:r], s12[:, l:r], u[:, l:r]),
                lambda l, r: nc.gpsimd.tensor_sub(cov[:, l:r], s12[:, l:r], u[:, l:r]),
                COV_DVE, fs,
            )
            # reuses u's slot (u is dead after cov) to stay inside SBUF
            m0 = post_pool.tile([P, FSMAX], bf16, tag="u", name="m0")
            split_op(
                lambda l, r: nc.vector.tensor_mul(
                    m0[:, l:r], r_pair[:, 0, l:r], r_pair[:, 1, l:r]
                ),
                lambda l, r: nc.gpsimd.tensor_mul(
                    m0[:, l:r], r_pair[:, 0, l:r], r_pair[:, 1, l:r]
                ),
                M0_DVE, fs,
            )
            corr = corr_pool.tile([P, FSMAX], bf16, tag=f"corr{bt}")
            split_op(
                lambda l, r: nc.vector.tensor_mul(corr[:, l:r], cov[:, l:r], m0[:, l:r]),
                lambda l, r: nc.gpsimd.tensor_mul(corr[:, l:r], cov[:, l:r], m0[:, l:r]),
                CORR_DVE, fs,
            )
            return corr

        def mean_store_stage(c, corrs):
            fs = CHUNKS[c]
            c0 = COFF[c]
            srow = row_pool.tile([1, FSMAX], bf16, tag="srow")
            for (l, r) in _slices(fs, SL):
                bps = b_psum.tile([1, SL], f32, tag="bps")
                for bt in range(NBT):
                    nc.tensor.matmul(
                        bps[:, 0 : r - l], bcol[:], corrs[bt][:, l:r],
                        start=(bt == 0), stop=(bt == NBT - 1),
                    )
                nc.scalar.activation(srow[:, l:r], bps[:, 0 : r - l], AOT.Copy)

            nsl = len(_slices(fs, SL))
            rsplit = int(nsl * RELU_ACT)
            for bt in range(NBT):
                b0 = bt * P
                outt = out_pool.tile([P, FSMAX], bf16, tag="outt")
                for si, (l, r) in enumerate(_slices(fs, SL)):
                    fps = f_psum.tile([P, SL], f32, tag="fps")
                    nc.tensor.matmul(
                        fps[:, 0 : r - l], ident[0][:], corrs[bt][:, l:r],
                        start=True, stop=False,
                    )
                    nc.tensor.matmul(
                        fps[:, 0 : r - l], negrow[:], srow[:, l:r],
                        start=False, stop=True,
                    )
                    if si < rsplit:
                        nc.scalar.activation(
                            outt[:, l:r], fps[:, 0 : r - l], AOT.Relu
                        )
                    else:
                        nc.vector.tensor_scalar_max(
                            outt[:, l:r], fps[:, 0 : r - l], 0.0
                        )
                nc.sync.dma_start(
                    out=out[b0 : b0 + P, c0 : c0 + fs], in_=outt[:, 0:fs]
                )

        tasks = [(c, bt) for c in range(NSC) for bt in range(NBT)]
        scans_q = {}
        corrs_q = {}
        for i in range(len(tasks) + 1):
            if i < len(tasks):
                scans_q[tasks[i]] = scan_stage(*tasks[i])
            if i > 0:
                c, bt = tasks[i - 1]
                corrs_q[(c, bt)] = post_stage(c, bt, scans_q.pop(tasks[i - 1]))
                if bt == NBT - 1:
                    mean_store_stage(c, [corrs_q.pop((c, b)) for b in range(NBT)])


def build_nc():
    from concourse import bacc

    nc = bacc.Bacc("TRN2", target_bir_lowering=False, debug=False, num_devices=NCORES)
    xs = nc.dram_tensor("xs", [B, CH, FIN], f32, kind="ExternalInput").ap()
    wconst = nc.dram_tensor("wconst", [CH, P, P], bf16, kind="ExternalInput").ap()
    out = nc.dram_tensor("out", [B, NLOC], bf16, kind="ExternalOutput").ap()
    with tile.TileContext(nc) as tc:
        _kernel_body(tc, out, xs, wconst)
    nc.compile()
    return nc


_NC = None


def _get_nc():
    global _NC
    if _NC is None:
        _NC = build_nc()
    return _NC


def make_in_maps(x):
    import ml_dtypes

    x = np.asarray(x, dtype=np.float32)
    xpad = np.zeros((B, CH, TPAD), dtype=np.float32)
    xpad[:, :, :T] = x
    eye = np.eye(P, dtype=np.float32)
    wconst = np.stack([eye, -eye]).astype(ml_dtypes.bfloat16)
    return [
        {
            "xs": np.ascontiguousarray(xpad[:, :, c * NLOC : c * NLOC + FIN]),
            "wconst": wconst,
        }
        for c in range(NCORES)
    ]


def _run(x, **kwargs):
    nc = _get_nc()
    res = run_bass_kernel_spmd(nc, make_in_maps(x), core_ids=list(range(NCORES)), **kwargs)
    outs = [np.asarray(res.results[c]["out"]) for c in range(NCORES)]
    full = np.concatenate(outs, axis=1)[:, :N].astype(np.float32)
    return full, res


def kernel(x):
    full, _ = _run(x)
    return full
